# revision 49
# baseline (speedup 1.0000x reference)
"""STFT (n_fft=4096, hop=1024, centered reflect-pad, Hann) on 8 TRN2 cores.

Algorithm: 2-stage Cooley-Tukey, n = 128*n1 + n2 (n1 in [0,32), n2 in [0,128)),
k = k1 + 32*k2 (k1 in [0,32), k2 in [0,64] for the 2049 kept bins).

  X[k1+32k2, b] = sum_n2 G[n2,k] * sum_n1 e^{-2pi i n1 k1/32} * xw[b, 128n1+n2]

Stage 1 runs frames-as-weights so its output lands transposed (n2 on
partitions): per 4-frame subgroup one [128,128] fp16 lhsT (4 frames
interleaved across partitions) against a constant one-hot-structured rhs
[128,256]. Stage 2 contracts n2 (K=128) with per-k1 twiddle matrices in fp16
over B=256-frame groups (N=256 matmuls).

DMA layout: the host pre-windows + pre-gathers the stage-1 lhsT content into
xfr[2, 128, 129*128] fp16 (partition p = 32j+8r+i, cols = 128*subgroup + m),
so every input DMA is a plain 2D tile load with multi-KB contiguous
per-partition runs (the fp32 4-copy scheme moved everything in 512B packets).
Output is written in SBUF order to o[128, 16512] fp16 (partition = 64c+p,
cols = 32*gb0 + q*B + b, bin = 32p+q) and unscrambled on the host.

Sharding: frame-parallel. Core i computes 516 frames starting at frame 512*i
(SPMD, same NEFF); host trims/concatenates to the 4097 global frames.
"""

import numpy as np

import concourse.bacc as bacc
import concourse.tile as tile
import concourse.mybir as mybir
from concourse import bass_utils

N_FFT = 4096
HOP = 1024
T = 4194304
NBINS = N_FFT // 2 + 1          # 2049
F_TOTAL = T // HOP + 1          # 4097
NCORES = 8

NF = 516                        # frames computed per core (129 subgroups of 4)
NSG = NF // 4                   # 129 subgroups
GROUPS = [256, 260]
L = (NF - 1) * HOP + N_FFT      # per-core input samples per plane = 531456

F32 = mybir.dt.float32
F16 = mybir.dt.float16

_cache = {}
LAST_EXEC_NS = None
LAST_RES = None


def _host_constants():
    n1 = np.arange(32)
    k1 = np.arange(32)
    C = np.cos(2 * np.pi * np.outer(n1, k1) / 32)
    S = np.sin(2 * np.pi * np.outer(n1, k1) / 32)
    R1 = np.concatenate([C, -S], axis=1)      # [n1, 64]
    R2 = np.concatenate([S, C], axis=1)
    # lhsT partition p = 32j + 8r + i  <->  (n1 = 8j+i, frame r)
    # column order (c, r): col = 4*c + r, so stage-1 PSUM comes out
    # slot-major and the PSUM->SBUF copy writes contiguous frame runs.
    R1D = np.zeros((128, 256), np.float16)
    R2D = np.zeros((128, 256), np.float16)
    for j in range(4):
        for i in range(8):
            for r in range(4):
                p = 32 * j + 8 * r + i
                R1D[p, r::4] = R1[8 * j + i]
                R2D[p, r::4] = R2[8 * j + i]

    n2 = np.arange(128)
    k2 = np.arange(64)
    Gp = np.zeros((128, 32 * 128), np.float16)
    Gq = np.zeros((128, 32 * 128), np.float16)
    for q in range(32):
        kk = q + 32 * k2
        ang = 2 * np.pi * np.outer(n2, kk) / N_FFT
        gr = np.cos(ang)
        gi = -np.sin(ang)
        Gp[:, 128 * q:128 * q + 64] = gr.astype(np.float16)
        Gp[:, 128 * q + 64:128 * q + 128] = gi.astype(np.float16)
        Gq[:, 128 * q:128 * q + 64] = (-gi).astype(np.float16)
        Gq[:, 128 * q + 64:128 * q + 128] = gr.astype(np.float16)

    alt = ((-1.0) ** n2).astype(np.float16)
    E1 = np.zeros((128, 2), np.float16)
    E2 = np.zeros((128, 2), np.float16)
    E1[:, 0] = alt
    E2[:, 1] = alt
    return (R1D, R2D, Gp, Gq, E1, E2)


def _build(stages=("dma", "s1", "s2", "out")):
    stages = set(stages)
    nc = bacc.Bacc("TRN2", target_bir_lowering=False, debug=False,
                   enable_asserts=False, num_devices=NCORES)
    xfr = nc.dram_tensor("xfr", [2, 128, NSG * 128], F16, kind="ExternalInput")
    # boot = [r1|r2 (512) | plane0 sg0-3 (512) | plane1 sg0-3 (512)]: one DMA
    # covers everything the first stage-1 block needs.
    boot = nc.dram_tensor("boot", [128, 1536], F16, kind="ExternalInput")
    gp = nc.dram_tensor("gp", [128, 32 * 128], F16, kind="ExternalInput")
    gq = nc.dram_tensor("gq", [128, 32 * 128], F16, kind="ExternalInput")
    e1 = nc.dram_tensor("e1", [128, 2], F16, kind="ExternalInput")
    e2 = nc.dram_tensor("e2", [128, 2], F16, kind="ExternalInput")
    out = nc.dram_tensor("o", [128, 32 * NF], F16, kind="ExternalOutput")
    oute = nc.dram_tensor("oe", [2, NF], F16, kind="ExternalOutput")

    with tile.TileContext(nc) as tc:
        with (
            tc.tile_pool(name="const", bufs=1) as cpool,
            tc.tile_pool(name="fr", bufs=4) as frpool,
            tc.tile_pool(name="ys", bufs=2) as yspool,
            tc.tile_pool(name="ost", bufs=2) as ostpool,
            tc.tile_pool(name="ps1", bufs=2, space="PSUM") as ps1pool,
            tc.tile_pool(name="ps2", bufs=3, space="PSUM") as ps2pool,
            tc.tile_pool(name="pse", bufs=1, space="PSUM") as psepool,
        ):
            t_boot = cpool.tile([128, 1536], F16, tag="boot")
            t_gp = cpool.tile([128, 32 * 128], F16, tag="gp")
            t_gq = cpool.tile([128, 32 * 128], F16, tag="gq")
            t_e1 = cpool.tile([128, 2], F16, tag="e1")
            t_e2 = cpool.tile([128, 2], F16, tag="e2")
            # boot (needed by the first stage-1 block) goes first on the sync
            # queue. The big stage-2 constants are interjected into the same
            # queue between input chunks — a concurrent queue starves the
            # input DMA.
            nc.sync.dma_start(t_boot[:], boot.ap()[:, :])
            t_r1 = t_boot[:, 0:256]
            t_r2 = t_boot[:, 256:512]

            # split by need-time so each piece slots between input chunks
            # without starving them: halves for q0-15 land late in group 0's
            # stream, halves for q16-31 ride group 1's stream.
            const_g0 = {
                3: lambda: nc.sync.dma_start(t_gp[:, 0:2048],
                                             gp.ap()[:, 0:2048]),
                4: lambda: (nc.sync.dma_start(t_gq[:, 0:2048],
                                              gq.ap()[:, 0:2048]),
                            nc.sync.dma_start(t_e1[:], e1.ap()[:, :]),
                            nc.sync.dma_start(t_e2[:], e2.ap()[:, :])),
            }
            const_g1 = {
                0: lambda: nc.sync.dma_start(t_gp[:, 2048:4096],
                                             gp.ap()[:, 2048:4096]),
                1: lambda: nc.sync.dma_start(t_gq[:, 2048:4096],
                                             gq.ap()[:, 2048:4096]),
            }

            starts = []
            gb0 = 0
            for B in GROUPS:
                starts.append(gb0)
                gb0 += B

            cp_parity = [0]

            def copy_op(dst, src):
                if cp_parity[0] % 2 == 0:
                    nc.vector.tensor_copy(dst, src)
                else:
                    nc.scalar.copy(dst, src)
                cp_parity[0] += 1

            def s1_block(ys_v, b0, ns, slc_r, slc_i):
                """One ps1 block: ns subgroups x 2 MMs + 1 copy."""
                ps1 = ps1pool.tile([128, 1024], F32, tag="ps1")
                for t in range(ns):
                    cs = 256 * t
                    nc.tensor.matmul(ps1[:, cs:cs + 256], slc_r(t),
                                     t_r1, start=True, stop=False)
                    nc.tensor.matmul(ps1[:, cs:cs + 256], slc_i(t),
                                     t_r2, start=False, stop=True)
                # ps1 col = 256*s_local + 4*c + r; ys col = c*B + b,
                # b = b0 + 4*s_local + r: 32B-contiguous dst runs
                src = ps1[:, 0:256 * ns].rearrange(
                    "p (s c r) -> p c s r", c=64, r=4)
                dstc = ys_v[:, :, b0:b0 + 4 * ns].rearrange(
                    "p c (s r) -> p c s r", r=4)
                copy_op(dstc, src)

            def gen_load_s1(gb0, B, ys_out, interject=None):
                """Yield once per ps1 block (8 MMs + 1 copy)."""
                nsub = B // 4
                sg0 = gb0 // 4
                # ys layout: col = c*B + b (slot-major) so stage-2 rhs
                # slices are contiguous in b.
                ys = yspool.tile([128, 64 * 260], F16, tag="ys")
                ys_out.append(ys)
                ys_v = ys[:, 0:64 * B].rearrange("p (c b) -> p c b", b=B)
                c0 = 0
                if gb0 == 0:
                    # subgroups 0-3 ride the boot tile
                    s1_block(ys_v, 0, 4,
                             lambda t: t_boot[:, 512 + 128 * t:640 + 128 * t],
                             lambda t: t_boot[:, 1024 + 128 * t:1152 + 128 * t])
                    yield
                    c0 = 4
                # chunk sizes ramp up so the first MMs start early
                chunks, rem = [], nsub - c0
                while rem > 0:
                    n = (4 if not chunks else 8) if gb0 == 0 and \
                        len(chunks) < 2 else 16
                    chunks.append(min(n, rem))
                    rem -= chunks[-1]
                for ci, ncs in enumerate(chunks):
                    fr = frpool.tile([128, 4096], F16, tag="fr")
                    a = 128 * (sg0 + c0)
                    dst = fr[:, 0:4096].rearrange(
                        "p (c m) -> p c m", c=2)[:, :, 0:128 * ncs]
                    src = xfr.ap()[:, :, a:a + 128 * ncs].rearrange(
                        "c p m -> p c m")
                    nc.sync.dma_start(dst, src)
                    if interject is not None and ci in interject:
                        interject[ci]()
                    if "s1" not in stages:
                        c0 += ncs
                        continue
                    for blk in range(0, ncs, 4):
                        ns = min(4, ncs - blk)
                        s1_block(ys_v, 4 * (c0 + blk), ns,
                                 lambda t, blk=blk: fr[
                                     :, 128 * (blk + t):128 * (blk + t) + 128],
                                 lambda t, blk=blk: fr[
                                     :, 2048 + 128 * (blk + t):
                                     2176 + 128 * (blk + t)])
                        yield
                    c0 += ncs

            def gen_s2_out(gb0, B, ys_h, qstep):
                """Yield once per q-unit (qstep q's: 2*qstep MMs + 1 copy).
                qstep=2 requires B <= 256 (two q's share one PSUM bank)."""
                if "s2" not in stages:
                    return
                ys = ys_h[0]
                ost = ostpool.tile([128, 32 * 260], F16, tag="ost")
                oc0 = 32 * gb0          # output col base for this group

                # bin 2048 (k1=0, k2=64) — emitted first to keep it off
                # the tail
                pse = psepool.tile([2, 260], F32, tag="pse")
                nc.tensor.matmul(pse[:, 0:B], t_e1[:], ys[:, 0:B],
                                 start=True, stop=False)
                nc.tensor.matmul(pse[:, 0:B], t_e2[:], ys[:, 32 * B:33 * B],
                                 start=False, stop=True)
                oste = ostpool.tile([2, 260], F16, tag="oste")
                nc.vector.tensor_copy(oste[:, 0:B], pse[:, 0:B])
                if "out" in stages:
                    nc.sync.dma_start(oute.ap()[:, gb0:gb0 + B],
                                      oste[:, 0:B])

                for q0 in range(0, 32, qstep):
                    ps2 = ps2pool.tile([128, 512], F32, tag="ps2")
                    for t in range(qstep):
                        q = q0 + t
                        rhs_r = ys[:, B * q:B * q + B]
                        rhs_i = ys[:, B * (32 + q):B * (32 + q) + B]
                        cs = 256 * t
                        nc.tensor.matmul(ps2[:, cs:cs + B],
                                         t_gp[:, 128 * q:128 * q + 128],
                                         rhs_r, start=True, stop=False)
                        nc.tensor.matmul(ps2[:, cs:cs + B],
                                         t_gq[:, 128 * q:128 * q + 128],
                                         rhs_i, start=False, stop=True)
                    dst = ost[:, B * q0:B * q0 + qstep * B]
                    if qstep == 2 and B == 256:
                        copy_op(dst, ps2[:, 0:512])
                    else:
                        copy_op(dst, ps2[:, 0:B])
                    qe = q0 + qstep         # q's finished
                    if "out" in stages:
                        if qe % 4 == 0 and qe <= 28:
                            a, w = B * (qe - 4), 4 * B
                        elif qe == 30:
                            a, w = 28 * B, 2 * B
                        else:
                            a = None
                        if a is not None:
                            nc.sync.dma_start(
                                out.ap()[:, oc0 + a:oc0 + a + w],
                                ost[:, a:a + w])
                    yield

                if "out" in stages:
                    a = 30 * B
                    nc.sync.dma_start(
                        out.ap()[:, oc0 + a:oc0 + a + 2 * B],
                        ost[:, a:a + 2 * B])

            # emission schedule: s1(g0) | s1(g1) interleaved with s2(g0)
            # (1 block : 1 paired-q unit) | s2(g1).
            h0, h1 = [], []
            for _ in gen_load_s1(starts[0], GROUPS[0], h0,
                                 interject=const_g0):
                pass
            g1 = gen_load_s1(starts[1], GROUPS[1], h1, interject=const_g1)
            g2 = gen_s2_out(starts[0], GROUPS[0], h0, qstep=2)
            while True:
                try:
                    next(g1)
                except StopIteration:
                    break
                next(g2, None)
            for _ in g2:
                pass
            for _ in gen_s2_out(starts[1], GROUPS[1], h1, qstep=1):
                pass

    nc.compile()
    return nc


def _prep_inputs(x, window):
    """Per-core stage-1 lhsT tensors: xfr[2, 128, 129*128] fp16 with
    partition p = 32j+8r+i holding frame-quarter j of frame 4*sg+r,
    cols = 128*sg + m, value = xp[1024*(b+j)+128i+m] * w[1024j+128i+m]."""
    pad = N_FFT // 2
    xp = np.pad(np.asarray(x), ((0, 0), (pad, pad)), mode="reflect")
    total = xp.shape[1]
    need = (NCORES - 1) * 512 * HOP + L
    xp_ext = np.zeros((2, max(total, need)), np.float32)
    xp_ext[:, :total] = xp
    w = np.asarray(window, np.float32)

    xfrs = []
    for i in range(NCORES):
        s0 = i * 512 * HOP
        seg = xp_ext[:, s0:s0 + L]
        xfr = np.empty((2, 128, NSG * 128), np.float16)
        for c in range(2):
            for j in range(4):
                Q = seg[c, 1024 * j:1024 * j + 1024 * NF].reshape(NF, 1024)
                Q = Q * w[1024 * j:1024 * (j + 1)][None, :]
                # [f, 1024] -> [sg, r, i, m] -> [r, i, sg, m]
                Q = Q.reshape(NSG, 4, 8, 128).transpose(1, 2, 0, 3)
                xfr[c, 32 * j:32 * j + 32] = \
                    Q.reshape(32, NSG * 128).astype(np.float16)
        xfrs.append(xfr)
    return xfrs


def kernel(x, window):
    import time
    t0 = time.time()
    x = np.asarray(x, np.float32)
    window = np.asarray(window, np.float32)
    if "nc" not in _cache:
        _cache["nc"] = _build()
    nc = _cache["nc"]
    print(f"[kernel] build done {time.time()-t0:.2f}s", flush=True)

    xfrs = _prep_inputs(x, window)
    R1D, R2D, Gp, Gq, E1, E2 = _host_constants()

    R12 = np.concatenate([R1D, R2D], axis=1)
    in_maps = []
    for i in range(NCORES):
        bt = np.concatenate(
            [R12, xfrs[i][0, :, 0:512], xfrs[i][1, :, 0:512]], axis=1)
        in_maps.append({"xfr": xfrs[i], "boot": bt,
                        "gp": Gp, "gq": Gq, "e1": E1, "e2": E2})

    print(f"[kernel] inputs prepped {time.time()-t0:.2f}s", flush=True)
    res = bass_utils.run_bass_kernel_spmd(nc, in_maps,
                                          core_ids=list(range(NCORES)))
    print(f"[kernel] spmd done {time.time()-t0:.2f}s", flush=True)
    global LAST_EXEC_NS, LAST_RES
    LAST_RES = res
    if res.exec_time_ns is not None:
        LAST_EXEC_NS = res.exec_time_ns
        print(f"[kernel] exec_time_ns={res.exec_time_ns}", flush=True)
        if res.instructions_and_trace is not None:
            print(f"[kernel] trace={res.instructions_and_trace[1]}",
                  flush=True)

    out = np.zeros((2, NBINS, F_TOTAL), np.float32)
    for i in range(NCORES):
        o = res.results[i]["o"]            # [128, 32*NF] fp16
        oe = res.results[i]["oe"]          # [2, NF] fp16
        f0 = 512 * i
        nf = 513 if i == NCORES - 1 else 512
        full = np.empty((2, 2048, NF), np.float32)
        gb0 = 0
        for B in GROUPS:
            seg = o[:, 32 * gb0:32 * gb0 + 32 * B].astype(np.float32)
            # [128, 32*B] -> [c, p, q, b] -> [c, 32p+q, b]
            full[:, :, gb0:gb0 + B] = \
                seg.reshape(2, 64, 32, B).reshape(2, 2048, B)
            gb0 += B
        out[:, :2048, f0:f0 + nf] = full[:, :, :nf]
        out[:, 2048, f0:f0 + nf] = oe[:, :nf].astype(np.float32)
    return out


# revision 53
# speedup vs baseline: 1.1520x; 1.1520x over previous
"""STFT (n_fft=4096, hop=1024, centered reflect-pad, Hann) on 8 TRN2 cores.

Algorithm: 2-stage Cooley-Tukey, n = 128*n1 + n2 (n1 in [0,32), n2 in [0,128)),
k = k1 + 32*k2 (k1 in [0,32), k2 in [0,64] for the 2049 kept bins).

  X[k1+32k2, b] = sum_n2 G[n2,k] * sum_n1 e^{-2pi i n1 k1/32} * xw[b, 128n1+n2]

Stage 1 runs frames-as-weights so its output lands transposed (n2 on
partitions): per 4-frame subgroup one [128,128] fp16 lhsT (4 frames
interleaved across partitions) against a constant one-hot-structured rhs
[128,256]. Stage 2 contracts n2 (K=128) with per-k1 twiddle matrices in fp16
over B=256-frame groups (N=256 matmuls).

DMA layout: the host pre-windows + pre-gathers the stage-1 lhsT content into
xfr[2, 128, 129*128] fp16 (partition p = 32j+8r+i, cols = 128*subgroup + m),
so every input DMA is a plain 2D tile load with multi-KB contiguous
per-partition runs (the fp32 4-copy scheme moved everything in 512B packets).
Output is written in SBUF order to o[128, 16512] fp16 (partition = 64c+p,
cols = 32*gb0 + q*B + b, bin = 32p+q) and unscrambled on the host.

Sharding: frame-parallel. Core i computes 516 frames starting at frame 512*i
(SPMD, same NEFF); host trims/concatenates to the 4097 global frames.
"""

import numpy as np

import concourse.bacc as bacc
import concourse.tile as tile
import concourse.mybir as mybir
from concourse import bass_utils

N_FFT = 4096
HOP = 1024
T = 4194304
NBINS = N_FFT // 2 + 1          # 2049
F_TOTAL = T // HOP + 1          # 4097
NCORES = 8

NF = 516                        # frames computed per core (129 subgroups of 4)
NSG = NF // 4                   # 129 subgroups
GROUPS = [260, 256]
L = (NF - 1) * HOP + N_FFT      # per-core input samples per plane = 531456

F32 = mybir.dt.float32
F16 = mybir.dt.float16

_cache = {}
LAST_EXEC_NS = None
LAST_RES = None


def _host_constants():
    n1 = np.arange(32)
    k1 = np.arange(32)
    C = np.cos(2 * np.pi * np.outer(n1, k1) / 32)
    S = np.sin(2 * np.pi * np.outer(n1, k1) / 32)
    R1 = np.concatenate([C, -S], axis=1)      # [n1, 64]
    R2 = np.concatenate([S, C], axis=1)
    # lhsT partition p = 32j + 8r + i  <->  (n1 = 8j+i, frame r)
    # column order (c, r): col = 4*c + r, so stage-1 PSUM comes out
    # slot-major and the PSUM->SBUF copy writes contiguous frame runs.
    R1D = np.zeros((128, 256), np.float16)
    R2D = np.zeros((128, 256), np.float16)
    for j in range(4):
        for i in range(8):
            for r in range(4):
                p = 32 * j + 8 * r + i
                R1D[p, r::4] = R1[8 * j + i]
                R2D[p, r::4] = R2[8 * j + i]

    n2 = np.arange(128)
    k2 = np.arange(64)
    Gp = np.zeros((128, 32 * 128), np.float16)
    Gq = np.zeros((128, 32 * 128), np.float16)
    for q in range(32):
        kk = q + 32 * k2
        ang = 2 * np.pi * np.outer(n2, kk) / N_FFT
        gr = np.cos(ang)
        gi = -np.sin(ang)
        Gp[:, 128 * q:128 * q + 64] = gr.astype(np.float16)
        Gp[:, 128 * q + 64:128 * q + 128] = gi.astype(np.float16)
        Gq[:, 128 * q:128 * q + 64] = (-gi).astype(np.float16)
        Gq[:, 128 * q + 64:128 * q + 128] = gr.astype(np.float16)

    alt = ((-1.0) ** n2).astype(np.float16)
    E1 = np.zeros((128, 2), np.float16)
    E2 = np.zeros((128, 2), np.float16)
    E1[:, 0] = alt
    E2[:, 1] = alt
    return (R1D, R2D, Gp, Gq, E1, E2)


def _build(stages=("dma", "s1", "s2", "out")):
    stages = set(stages)
    nc = bacc.Bacc("TRN2", target_bir_lowering=False, debug=False,
                   enable_asserts=False, num_devices=NCORES)
    xfr = nc.dram_tensor("xfr", [2, 128, NSG * 128], F16, kind="ExternalInput")
    # boot = [r1|r2 (512) | plane0 sg0-3 (512) | plane1 sg0-3 (512)]: one DMA
    # covers everything the first stage-1 block needs.
    boot = nc.dram_tensor("boot", [128, 1536], F16, kind="ExternalInput")
    gp = nc.dram_tensor("gp", [128, 32 * 128], F16, kind="ExternalInput")
    gq = nc.dram_tensor("gq", [128, 32 * 128], F16, kind="ExternalInput")
    e1 = nc.dram_tensor("e1", [128, 2], F16, kind="ExternalInput")
    e2 = nc.dram_tensor("e2", [128, 2], F16, kind="ExternalInput")
    out = nc.dram_tensor("o", [128, 32 * NF], F16, kind="ExternalOutput")
    oute = nc.dram_tensor("oe", [2, NF], F16, kind="ExternalOutput")

    with tile.TileContext(nc) as tc:
        with (
            tc.tile_pool(name="const", bufs=1) as cpool,
            tc.tile_pool(name="fr", bufs=4) as frpool,
            tc.tile_pool(name="ys", bufs=2) as yspool,
            tc.tile_pool(name="ost", bufs=2) as ostpool,
            tc.tile_pool(name="ps1", bufs=2, space="PSUM") as ps1pool,
            tc.tile_pool(name="ps2", bufs=3, space="PSUM") as ps2pool,
            tc.tile_pool(name="pse", bufs=1, space="PSUM") as psepool,
        ):
            t_boot = cpool.tile([128, 1536], F16, tag="boot")
            t_gp = cpool.tile([128, 32 * 128], F16, tag="gp")
            t_gq = cpool.tile([128, 32 * 128], F16, tag="gq")
            t_e1 = cpool.tile([128, 2], F16, tag="e1")
            t_e2 = cpool.tile([128, 2], F16, tag="e2")
            # boot (needed by the first stage-1 block) goes first on the sync
            # queue. The big stage-2 constants are interjected into the same
            # queue between input chunks — a concurrent queue starves the
            # input DMA.
            nc.sync.dma_start(t_boot[:], boot.ap()[:, :])
            t_r1 = t_boot[:, 0:256]
            t_r2 = t_boot[:, 256:512]

            # split so each piece slots between input chunks without
            # starving them
            const_g0 = {
                2: lambda: nc.sync.dma_start(t_gp[:, 0:2048],
                                             gp.ap()[:, 0:2048]),
                3: lambda: nc.sync.dma_start(t_gp[:, 2048:4096],
                                             gp.ap()[:, 2048:4096]),
                4: lambda: nc.sync.dma_start(t_gq[:, 0:2048],
                                             gq.ap()[:, 0:2048]),
                5: lambda: (nc.sync.dma_start(t_gq[:, 2048:4096],
                                              gq.ap()[:, 2048:4096]),
                            nc.sync.dma_start(t_e1[:], e1.ap()[:, :]),
                            nc.sync.dma_start(t_e2[:], e2.ap()[:, :])),
            }

            # HAM throttle warmup: the PE boots clock-gated (K=4/8) and only
            # activity releases it. These dummy MMs (garbage in, PSUM never
            # read) run during the boot-DMA wait so the real stream starts
            # at full clock.
            warm_sb = cpool.tile([128, 2], F16, tag="warm")
            nc.vector.memset(warm_sb[:], 0.0)
            warm_ps = psepool.tile([2, 260], F32, tag="pse")
            for _ in range(30):
                nc.tensor.matmul(warm_ps[:, 0:2], warm_sb[:], warm_sb[:],
                                 start=True, stop=True)

            starts = []
            gb0 = 0
            for B in GROUPS:
                starts.append(gb0)
                gb0 += B

            cp_parity = [0]

            def copy_op(dst, src):
                if cp_parity[0] % 2 == 0:
                    nc.vector.tensor_copy(dst, src)
                else:
                    nc.scalar.copy(dst, src)
                cp_parity[0] += 1

            def s1_block(ys_v, b0, ns, slc_r, slc_i):
                """One ps1 block: ns subgroups x 2 MMs + 1 copy."""
                ps1 = ps1pool.tile([128, 1024], F32, tag="ps1")
                for t in range(ns):
                    cs = 256 * t
                    nc.tensor.matmul(ps1[:, cs:cs + 256], slc_r(t),
                                     t_r1, start=True, stop=False)
                    nc.tensor.matmul(ps1[:, cs:cs + 256], slc_i(t),
                                     t_r2, start=False, stop=True)
                # ps1 col = 256*s_local + 4*c + r; ys col = c*B + b,
                # b = b0 + 4*s_local + r: 32B-contiguous dst runs
                src = ps1[:, 0:256 * ns].rearrange(
                    "p (s c r) -> p c s r", c=64, r=4)
                dstc = ys_v[:, :, b0:b0 + 4 * ns].rearrange(
                    "p c (s r) -> p c s r", r=4)
                copy_op(dstc, src)

            def gen_load_s1(gb0, B, ys_out, interject=None):
                """Yield once per ps1 block (8 MMs + 1 copy)."""
                nsub = B // 4
                sg0 = gb0 // 4
                # ys layout: col = c*B + b (slot-major) so stage-2 rhs
                # slices are contiguous in b.
                ys = yspool.tile([128, 64 * 260], F16, tag="ys")
                ys_out.append(ys)
                ys_v = ys[:, 0:64 * B].rearrange("p (c b) -> p c b", b=B)
                c0 = 0
                if gb0 == 0:
                    # subgroups 0-3 ride the boot tile
                    s1_block(ys_v, 0, 4,
                             lambda t: t_boot[:, 512 + 128 * t:640 + 128 * t],
                             lambda t: t_boot[:, 1024 + 128 * t:1152 + 128 * t])
                    yield
                    c0 = 4
                # chunk sizes ramp up so the first MMs start early
                chunks, rem = [], nsub - c0
                while rem > 0:
                    n = (4 if not chunks else 8) if gb0 == 0 and \
                        len(chunks) < 2 else 16
                    chunks.append(min(n, rem))
                    rem -= chunks[-1]
                for ci, ncs in enumerate(chunks):
                    fr = frpool.tile([128, 4096], F16, tag="fr")
                    a = 128 * (sg0 + c0)
                    dst = fr[:, 0:4096].rearrange(
                        "p (c m) -> p c m", c=2)[:, :, 0:128 * ncs]
                    src = xfr.ap()[:, :, a:a + 128 * ncs].rearrange(
                        "c p m -> p c m")
                    nc.sync.dma_start(dst, src)
                    if interject is not None and ci in interject:
                        interject[ci]()
                    if "s1" not in stages:
                        c0 += ncs
                        continue
                    for blk in range(0, ncs, 4):
                        ns = min(4, ncs - blk)
                        s1_block(ys_v, 4 * (c0 + blk), ns,
                                 lambda t, blk=blk: fr[
                                     :, 128 * (blk + t):128 * (blk + t) + 128],
                                 lambda t, blk=blk: fr[
                                     :, 2048 + 128 * (blk + t):
                                     2176 + 128 * (blk + t)])
                        yield
                    c0 += ncs

            def gen_s2_out(gb0, B, ys_h, qstep):
                """Yield once per q-unit (qstep q's: 2*qstep MMs + 1 copy).
                qstep=2 requires B <= 256 (two q's share one PSUM bank)."""
                if "s2" not in stages:
                    return
                ys = ys_h[0]
                ost = ostpool.tile([128, 32 * 260], F16, tag="ost")
                oc0 = 32 * gb0          # output col base for this group

                # bin 2048 (k1=0, k2=64) — emitted first to keep it off
                # the tail
                pse = psepool.tile([2, 260], F32, tag="pse")
                nc.tensor.matmul(pse[:, 0:B], t_e1[:], ys[:, 0:B],
                                 start=True, stop=False)
                nc.tensor.matmul(pse[:, 0:B], t_e2[:], ys[:, 32 * B:33 * B],
                                 start=False, stop=True)
                oste = ostpool.tile([2, 260], F16, tag="oste")
                nc.vector.tensor_copy(oste[:, 0:B], pse[:, 0:B])
                if "out" in stages:
                    nc.sync.dma_start(oute.ap()[:, gb0:gb0 + B],
                                      oste[:, 0:B])

                for q0 in range(0, 32, qstep):
                    ps2 = ps2pool.tile([128, 512], F32, tag="ps2")
                    for t in range(qstep):
                        q = q0 + t
                        rhs_r = ys[:, B * q:B * q + B]
                        rhs_i = ys[:, B * (32 + q):B * (32 + q) + B]
                        cs = 256 * t
                        nc.tensor.matmul(ps2[:, cs:cs + B],
                                         t_gp[:, 128 * q:128 * q + 128],
                                         rhs_r, start=True, stop=False)
                        nc.tensor.matmul(ps2[:, cs:cs + B],
                                         t_gq[:, 128 * q:128 * q + 128],
                                         rhs_i, start=False, stop=True)
                    dst = ost[:, B * q0:B * q0 + qstep * B]
                    if qstep == 2 and B == 256:
                        copy_op(dst, ps2[:, 0:512])
                    else:
                        copy_op(dst, ps2[:, 0:B])
                    qe = q0 + qstep         # q's finished
                    if "out" in stages:
                        if qe % 4 == 0 and qe <= 28:
                            a, w = B * (qe - 4), 4 * B
                        elif qe == 30:
                            a, w = 28 * B, 2 * B
                        else:
                            a = None
                        if a is not None:
                            nc.sync.dma_start(
                                out.ap()[:, oc0 + a:oc0 + a + w],
                                ost[:, a:a + w])
                    yield

                if "out" in stages:
                    a = 30 * B
                    nc.sync.dma_start(
                        out.ap()[:, oc0 + a:oc0 + a + 2 * B],
                        ost[:, a:a + 2 * B])

            # emission schedule: s1(g0) | s1(g1) interleaved with s2(g0)
            # (1 block : 2 q-units) | s2(g1) with paired-q copies.
            h0, h1 = [], []
            for _ in gen_load_s1(starts[0], GROUPS[0], h0,
                                 interject=const_g0):
                pass
            g1 = gen_load_s1(starts[1], GROUPS[1], h1)
            g2 = gen_s2_out(starts[0], GROUPS[0], h0, qstep=1)
            while True:
                try:
                    next(g1)
                except StopIteration:
                    break
                next(g2, None)
                next(g2, None)
            for _ in g2:
                pass
            for _ in gen_s2_out(starts[1], GROUPS[1], h1, qstep=2):
                pass

    nc.compile()
    return nc


def _prep_inputs(x, window):
    """Per-core stage-1 lhsT tensors: xfr[2, 128, 129*128] fp16 with
    partition p = 32j+8r+i holding frame-quarter j of frame 4*sg+r,
    cols = 128*sg + m, value = xp[1024*(b+j)+128i+m] * w[1024j+128i+m]."""
    pad = N_FFT // 2
    xp = np.pad(np.asarray(x), ((0, 0), (pad, pad)), mode="reflect")
    total = xp.shape[1]
    need = (NCORES - 1) * 512 * HOP + L
    xp_ext = np.zeros((2, max(total, need)), np.float32)
    xp_ext[:, :total] = xp
    w = np.asarray(window, np.float32)

    xfrs = []
    for i in range(NCORES):
        s0 = i * 512 * HOP
        seg = xp_ext[:, s0:s0 + L]
        xfr = np.empty((2, 128, NSG * 128), np.float16)
        for c in range(2):
            for j in range(4):
                Q = seg[c, 1024 * j:1024 * j + 1024 * NF].reshape(NF, 1024)
                Q = Q * w[1024 * j:1024 * (j + 1)][None, :]
                # [f, 1024] -> [sg, r, i, m] -> [r, i, sg, m]
                Q = Q.reshape(NSG, 4, 8, 128).transpose(1, 2, 0, 3)
                xfr[c, 32 * j:32 * j + 32] = \
                    Q.reshape(32, NSG * 128).astype(np.float16)
        xfrs.append(xfr)
    return xfrs


def kernel(x, window):
    import time
    t0 = time.time()
    x = np.asarray(x, np.float32)
    window = np.asarray(window, np.float32)
    if "nc" not in _cache:
        _cache["nc"] = _build()
    nc = _cache["nc"]
    print(f"[kernel] build done {time.time()-t0:.2f}s", flush=True)

    xfrs = _prep_inputs(x, window)
    R1D, R2D, Gp, Gq, E1, E2 = _host_constants()

    R12 = np.concatenate([R1D, R2D], axis=1)
    in_maps = []
    for i in range(NCORES):
        bt = np.concatenate(
            [R12, xfrs[i][0, :, 0:512], xfrs[i][1, :, 0:512]], axis=1)
        in_maps.append({"xfr": xfrs[i], "boot": bt,
                        "gp": Gp, "gq": Gq, "e1": E1, "e2": E2})

    print(f"[kernel] inputs prepped {time.time()-t0:.2f}s", flush=True)
    res = bass_utils.run_bass_kernel_spmd(nc, in_maps,
                                          core_ids=list(range(NCORES)))
    print(f"[kernel] spmd done {time.time()-t0:.2f}s", flush=True)
    global LAST_EXEC_NS, LAST_RES
    LAST_RES = res
    if res.exec_time_ns is not None:
        LAST_EXEC_NS = res.exec_time_ns
        print(f"[kernel] exec_time_ns={res.exec_time_ns}", flush=True)
        if res.instructions_and_trace is not None:
            print(f"[kernel] trace={res.instructions_and_trace[1]}",
                  flush=True)

    out = np.zeros((2, NBINS, F_TOTAL), np.float32)
    for i in range(NCORES):
        o = res.results[i]["o"]            # [128, 32*NF] fp16
        oe = res.results[i]["oe"]          # [2, NF] fp16
        f0 = 512 * i
        nf = 513 if i == NCORES - 1 else 512
        full = np.empty((2, 2048, NF), np.float32)
        gb0 = 0
        for B in GROUPS:
            seg = o[:, 32 * gb0:32 * gb0 + 32 * B].astype(np.float32)
            # [128, 32*B] -> [c, p, q, b] -> [c, 32p+q, b]
            full[:, :, gb0:gb0 + B] = \
                seg.reshape(2, 64, 32, B).reshape(2, 2048, B)
            gb0 += B
        out[:, :2048, f0:f0 + nf] = full[:, :, :nf]
        out[:, 2048, f0:f0 + nf] = oe[:, :nf].astype(np.float32)
    return out


# revision 55
# speedup vs baseline: 1.1722x; 1.0175x over previous
"""STFT (n_fft=4096, hop=1024, centered reflect-pad, Hann) on 8 TRN2 cores.

Algorithm: 2-stage Cooley-Tukey, n = 128*n1 + n2 (n1 in [0,32), n2 in [0,128)),
k = k1 + 32*k2 (k1 in [0,32), k2 in [0,64] for the 2049 kept bins).

  X[k1+32k2, b] = sum_n2 G[n2,k] * sum_n1 e^{-2pi i n1 k1/32} * xw[b, 128n1+n2]

Stage 1 runs frames-as-weights so its output lands transposed (n2 on
partitions): per 4-frame subgroup one [128,128] fp16 lhsT (4 frames
interleaved across partitions) against a constant one-hot-structured rhs
[128,256]. Stage 2 contracts n2 (K=128) with per-k1 twiddle matrices in fp16
over B=256-frame groups (N=256 matmuls).

DMA layout: the host pre-windows + pre-gathers the stage-1 lhsT content into
xfr[2, 128, 129*128] fp16 (partition p = 32j+8r+i, cols = 128*subgroup + m),
so every input DMA is a plain 2D tile load with multi-KB contiguous
per-partition runs (the fp32 4-copy scheme moved everything in 512B packets).
Output is written in SBUF order to o[128, 16512] fp16 (partition = 64c+p,
cols = 32*gb0 + q*B + b, bin = 32p+q) and unscrambled on the host.

Sharding: frame-parallel. Core i computes 516 frames starting at frame 512*i
(SPMD, same NEFF); host trims/concatenates to the 4097 global frames.
"""

import numpy as np

import concourse.bacc as bacc
import concourse.tile as tile
import concourse.mybir as mybir
from concourse import bass_utils

N_FFT = 4096
HOP = 1024
T = 4194304
NBINS = N_FFT // 2 + 1          # 2049
F_TOTAL = T // HOP + 1          # 4097
NCORES = 8

NF = 516                        # frames computed per core (129 subgroups of 4)
NSG = NF // 4                   # 129 subgroups
GROUPS = [260, 256]
L = (NF - 1) * HOP + N_FFT      # per-core input samples per plane = 531456

F32 = mybir.dt.float32
F16 = mybir.dt.float16

_cache = {}
LAST_EXEC_NS = None
LAST_RES = None


def _host_constants():
    n1 = np.arange(32)
    k1 = np.arange(32)
    C = np.cos(2 * np.pi * np.outer(n1, k1) / 32)
    S = np.sin(2 * np.pi * np.outer(n1, k1) / 32)
    R1 = np.concatenate([C, -S], axis=1)      # [n1, 64]
    R2 = np.concatenate([S, C], axis=1)
    # lhsT partition p = 32j + 8r + i  <->  (n1 = 8j+i, frame r)
    # column order (c, r): col = 4*c + r, so stage-1 PSUM comes out
    # slot-major and the PSUM->SBUF copy writes contiguous frame runs.
    R1D = np.zeros((128, 256), np.float16)
    R2D = np.zeros((128, 256), np.float16)
    for j in range(4):
        for i in range(8):
            for r in range(4):
                p = 32 * j + 8 * r + i
                R1D[p, r::4] = R1[8 * j + i]
                R2D[p, r::4] = R2[8 * j + i]

    n2 = np.arange(128)
    k2 = np.arange(64)
    Gp = np.zeros((128, 32 * 128), np.float16)
    Gq = np.zeros((128, 32 * 128), np.float16)
    for q in range(32):
        kk = q + 32 * k2
        ang = 2 * np.pi * np.outer(n2, kk) / N_FFT
        gr = np.cos(ang)
        gi = -np.sin(ang)
        Gp[:, 128 * q:128 * q + 64] = gr.astype(np.float16)
        Gp[:, 128 * q + 64:128 * q + 128] = gi.astype(np.float16)
        Gq[:, 128 * q:128 * q + 64] = (-gi).astype(np.float16)
        Gq[:, 128 * q + 64:128 * q + 128] = gr.astype(np.float16)

    alt = ((-1.0) ** n2).astype(np.float16)
    E1 = np.zeros((128, 2), np.float16)
    E2 = np.zeros((128, 2), np.float16)
    E1[:, 0] = alt
    E2[:, 1] = alt
    return (R1D, R2D, Gp, Gq, E1, E2)


def _build(stages=("dma", "s1", "s2", "out")):
    stages = set(stages)
    nc = bacc.Bacc("TRN2", target_bir_lowering=False, debug=False,
                   enable_asserts=False, num_devices=NCORES)
    xfr = nc.dram_tensor("xfr", [2, 128, NSG * 128], F16, kind="ExternalInput")
    # boot = [r1|r2 (512) | plane0 sg0-3 (512) | plane1 sg0-3 (512)]: one DMA
    # covers everything the first stage-1 block needs.
    boot = nc.dram_tensor("boot", [128, 1536], F16, kind="ExternalInput")
    gp = nc.dram_tensor("gp", [128, 32 * 128], F16, kind="ExternalInput")
    gq = nc.dram_tensor("gq", [128, 32 * 128], F16, kind="ExternalInput")
    e1 = nc.dram_tensor("e1", [128, 2], F16, kind="ExternalInput")
    e2 = nc.dram_tensor("e2", [128, 2], F16, kind="ExternalInput")
    out = nc.dram_tensor("o", [128, 32 * NF], F16, kind="ExternalOutput")
    oute = nc.dram_tensor("oe", [2, NF], F16, kind="ExternalOutput")

    with tile.TileContext(nc) as tc:
        with (
            tc.tile_pool(name="const", bufs=1) as cpool,
            tc.tile_pool(name="fr", bufs=6) as frpool,
            tc.tile_pool(name="ys", bufs=2) as yspool,
            tc.tile_pool(name="ost", bufs=2) as ostpool,
            tc.tile_pool(name="ps1", bufs=2, space="PSUM") as ps1pool,
            tc.tile_pool(name="ps2", bufs=3, space="PSUM") as ps2pool,
            tc.tile_pool(name="pse", bufs=1, space="PSUM") as psepool,
        ):
            t_boot = cpool.tile([128, 1536], F16, tag="boot")
            t_gp = cpool.tile([128, 32 * 128], F16, tag="gp")
            t_gq = cpool.tile([128, 32 * 128], F16, tag="gq")
            t_e1 = cpool.tile([128, 2], F16, tag="e1")
            t_e2 = cpool.tile([128, 2], F16, tag="e2")
            # boot (needed by the first stage-1 block) goes first on the sync
            # queue. The big stage-2 constants are interjected into the same
            # queue between input chunks — a concurrent queue starves the
            # input DMA.
            nc.sync.dma_start(t_boot[:], boot.ap()[:, :])
            t_r1 = t_boot[:, 0:256]
            t_r2 = t_boot[:, 256:512]

            # split so each piece slots between input chunks without
            # starving them
            const_g0 = {
                2: lambda: nc.sync.dma_start(t_gp[:, 0:2048],
                                             gp.ap()[:, 0:2048]),
                3: lambda: nc.sync.dma_start(t_gp[:, 2048:4096],
                                             gp.ap()[:, 2048:4096]),
                4: lambda: nc.sync.dma_start(t_gq[:, 0:2048],
                                             gq.ap()[:, 0:2048]),
                5: lambda: (nc.sync.dma_start(t_gq[:, 2048:4096],
                                              gq.ap()[:, 2048:4096]),
                            nc.sync.dma_start(t_e1[:], e1.ap()[:, :]),
                            nc.sync.dma_start(t_e2[:], e2.ap()[:, :])),
            }

            # HAM throttle warmup: the PE boots clock-gated (K=4/8) and only
            # activity releases it. These dummy MMs (garbage in, PSUM never
            # read) run during the boot-DMA wait so the real stream starts
            # at full clock.
            warm_sb = cpool.tile([128, 2], F16, tag="warm")
            nc.vector.memset(warm_sb[:], 0.0)
            warm_ps = psepool.tile([2, 260], F32, tag="pse")
            for _ in range(30):
                nc.tensor.matmul(warm_ps[:, 0:2], warm_sb[:], warm_sb[:],
                                 start=True, stop=True)

            starts = []
            gb0 = 0
            for B in GROUPS:
                starts.append(gb0)
                gb0 += B

            cp_parity = [0]

            def copy_op(dst, src):
                if cp_parity[0] % 2 == 0:
                    nc.vector.tensor_copy(dst, src)
                else:
                    nc.scalar.copy(dst, src)
                cp_parity[0] += 1

            def s1_block(ys_v, b0, ns, slc_r, slc_i):
                """One ps1 block: ns subgroups x 2 MMs + 1 copy."""
                ps1 = ps1pool.tile([128, 1024], F32, tag="ps1")
                for t in range(ns):
                    cs = 256 * t
                    nc.tensor.matmul(ps1[:, cs:cs + 256], slc_r(t),
                                     t_r1, start=True, stop=False)
                    nc.tensor.matmul(ps1[:, cs:cs + 256], slc_i(t),
                                     t_r2, start=False, stop=True)
                # ps1 col = 256*s_local + 4*c + r; ys col = c*B + b,
                # b = b0 + 4*s_local + r: 32B-contiguous dst runs
                src = ps1[:, 0:256 * ns].rearrange(
                    "p (s c r) -> p c s r", c=64, r=4)
                dstc = ys_v[:, :, b0:b0 + 4 * ns].rearrange(
                    "p c (s r) -> p c s r", r=4)
                copy_op(dstc, src)

            def gen_load_s1(gb0, B, ys_out, interject=None):
                """Yield once per ps1 block (8 MMs + 1 copy)."""
                nsub = B // 4
                sg0 = gb0 // 4
                # ys layout: col = c*B + b (slot-major) so stage-2 rhs
                # slices are contiguous in b.
                ys = yspool.tile([128, 64 * 260], F16, tag="ys")
                ys_out.append(ys)
                ys_v = ys[:, 0:64 * B].rearrange("p (c b) -> p c b", b=B)
                c0 = 0
                if gb0 == 0:
                    # subgroups 0-3 ride the boot tile
                    s1_block(ys_v, 0, 4,
                             lambda t: t_boot[:, 512 + 128 * t:640 + 128 * t],
                             lambda t: t_boot[:, 1024 + 128 * t:1152 + 128 * t])
                    yield
                    c0 = 4
                # chunk sizes ramp up so the first MMs start early; group 1
                # uses finer chunks to smooth queue bursts in the
                # interleaved section
                chunks, rem = [], nsub - c0
                while rem > 0:
                    if gb0 == 0:
                        n = (4 if not chunks else 8) if len(chunks) < 2 \
                            else 16
                    else:
                        n = 8
                    chunks.append(min(n, rem))
                    rem -= chunks[-1]
                for ci, ncs in enumerate(chunks):
                    fr = frpool.tile([128, 4096], F16, tag="fr")
                    a = 128 * (sg0 + c0)
                    dst = fr[:, 0:4096].rearrange(
                        "p (c m) -> p c m", c=2)[:, :, 0:128 * ncs]
                    src = xfr.ap()[:, :, a:a + 128 * ncs].rearrange(
                        "c p m -> p c m")
                    nc.sync.dma_start(dst, src)
                    if interject is not None and ci in interject:
                        interject[ci]()
                    if "s1" not in stages:
                        c0 += ncs
                        continue
                    for blk in range(0, ncs, 4):
                        ns = min(4, ncs - blk)
                        s1_block(ys_v, 4 * (c0 + blk), ns,
                                 lambda t, blk=blk: fr[
                                     :, 128 * (blk + t):128 * (blk + t) + 128],
                                 lambda t, blk=blk: fr[
                                     :, 2048 + 128 * (blk + t):
                                     2176 + 128 * (blk + t)])
                        yield
                    c0 += ncs

            def gen_s2_out(gb0, B, ys_h, qstep):
                """Yield once per q-unit (qstep q's: 2*qstep MMs + 1 copy).
                qstep=2 requires B <= 256 (two q's share one PSUM bank)."""
                if "s2" not in stages:
                    return
                ys = ys_h[0]
                ost = ostpool.tile([128, 32 * 260], F16, tag="ost")
                oc0 = 32 * gb0          # output col base for this group

                # bin 2048 (k1=0, k2=64) — emitted first to keep it off
                # the tail
                pse = psepool.tile([2, 260], F32, tag="pse")
                nc.tensor.matmul(pse[:, 0:B], t_e1[:], ys[:, 0:B],
                                 start=True, stop=False)
                nc.tensor.matmul(pse[:, 0:B], t_e2[:], ys[:, 32 * B:33 * B],
                                 start=False, stop=True)
                oste = ostpool.tile([2, 260], F16, tag="oste")
                nc.vector.tensor_copy(oste[:, 0:B], pse[:, 0:B])
                if "out" in stages:
                    nc.sync.dma_start(oute.ap()[:, gb0:gb0 + B],
                                      oste[:, 0:B])

                for q0 in range(0, 32, qstep):
                    ps2 = ps2pool.tile([128, 512], F32, tag="ps2")
                    for t in range(qstep):
                        q = q0 + t
                        rhs_r = ys[:, B * q:B * q + B]
                        rhs_i = ys[:, B * (32 + q):B * (32 + q) + B]
                        cs = 256 * t
                        nc.tensor.matmul(ps2[:, cs:cs + B],
                                         t_gp[:, 128 * q:128 * q + 128],
                                         rhs_r, start=True, stop=False)
                        nc.tensor.matmul(ps2[:, cs:cs + B],
                                         t_gq[:, 128 * q:128 * q + 128],
                                         rhs_i, start=False, stop=True)
                    dst = ost[:, B * q0:B * q0 + qstep * B]
                    if qstep == 2 and B == 256:
                        copy_op(dst, ps2[:, 0:512])
                    else:
                        copy_op(dst, ps2[:, 0:B])
                    qe = q0 + qstep         # q's finished
                    if "out" in stages:
                        if qe % 4 == 0 and qe <= 28:
                            a, w = B * (qe - 4), 4 * B
                        elif qe == 30:
                            a, w = 28 * B, 2 * B
                        else:
                            a = None
                        if a is not None:
                            nc.sync.dma_start(
                                out.ap()[:, oc0 + a:oc0 + a + w],
                                ost[:, a:a + w])
                    yield

                if "out" in stages:
                    a = 30 * B
                    nc.sync.dma_start(
                        out.ap()[:, oc0 + a:oc0 + a + 2 * B],
                        ost[:, a:a + 2 * B])

            # emission schedule: s1(g0) | s1(g1) interleaved with s2(g0)
            # (1 block : 2 q-units) | s2(g1) with paired-q copies.
            h0, h1 = [], []
            for _ in gen_load_s1(starts[0], GROUPS[0], h0,
                                 interject=const_g0):
                pass
            g1 = gen_load_s1(starts[1], GROUPS[1], h1)
            g2 = gen_s2_out(starts[0], GROUPS[0], h0, qstep=1)
            while True:
                try:
                    next(g1)
                except StopIteration:
                    break
                next(g2, None)
                next(g2, None)
            for _ in g2:
                pass
            for _ in gen_s2_out(starts[1], GROUPS[1], h1, qstep=2):
                pass

    nc.compile()
    return nc


def _prep_inputs(x, window):
    """Per-core stage-1 lhsT tensors: xfr[2, 128, 129*128] fp16 with
    partition p = 32j+8r+i holding frame-quarter j of frame 4*sg+r,
    cols = 128*sg + m, value = xp[1024*(b+j)+128i+m] * w[1024j+128i+m]."""
    pad = N_FFT // 2
    xp = np.pad(np.asarray(x), ((0, 0), (pad, pad)), mode="reflect")
    total = xp.shape[1]
    need = (NCORES - 1) * 512 * HOP + L
    xp_ext = np.zeros((2, max(total, need)), np.float32)
    xp_ext[:, :total] = xp
    w = np.asarray(window, np.float32)

    xfrs = []
    for i in range(NCORES):
        s0 = i * 512 * HOP
        seg = xp_ext[:, s0:s0 + L]
        xfr = np.empty((2, 128, NSG * 128), np.float16)
        for c in range(2):
            for j in range(4):
                Q = seg[c, 1024 * j:1024 * j + 1024 * NF].reshape(NF, 1024)
                Q = Q * w[1024 * j:1024 * (j + 1)][None, :]
                # [f, 1024] -> [sg, r, i, m] -> [r, i, sg, m]
                Q = Q.reshape(NSG, 4, 8, 128).transpose(1, 2, 0, 3)
                xfr[c, 32 * j:32 * j + 32] = \
                    Q.reshape(32, NSG * 128).astype(np.float16)
        xfrs.append(xfr)
    return xfrs


def kernel(x, window):
    import time
    t0 = time.time()
    x = np.asarray(x, np.float32)
    window = np.asarray(window, np.float32)
    if "nc" not in _cache:
        _cache["nc"] = _build()
    nc = _cache["nc"]
    print(f"[kernel] build done {time.time()-t0:.2f}s", flush=True)

    xfrs = _prep_inputs(x, window)
    R1D, R2D, Gp, Gq, E1, E2 = _host_constants()

    R12 = np.concatenate([R1D, R2D], axis=1)
    in_maps = []
    for i in range(NCORES):
        bt = np.concatenate(
            [R12, xfrs[i][0, :, 0:512], xfrs[i][1, :, 0:512]], axis=1)
        in_maps.append({"xfr": xfrs[i], "boot": bt,
                        "gp": Gp, "gq": Gq, "e1": E1, "e2": E2})

    print(f"[kernel] inputs prepped {time.time()-t0:.2f}s", flush=True)
    res = bass_utils.run_bass_kernel_spmd(nc, in_maps,
                                          core_ids=list(range(NCORES)))
    print(f"[kernel] spmd done {time.time()-t0:.2f}s", flush=True)
    global LAST_EXEC_NS, LAST_RES
    LAST_RES = res
    if res.exec_time_ns is not None:
        LAST_EXEC_NS = res.exec_time_ns
        print(f"[kernel] exec_time_ns={res.exec_time_ns}", flush=True)
        if res.instructions_and_trace is not None:
            print(f"[kernel] trace={res.instructions_and_trace[1]}",
                  flush=True)

    out = np.zeros((2, NBINS, F_TOTAL), np.float32)
    for i in range(NCORES):
        o = res.results[i]["o"]            # [128, 32*NF] fp16
        oe = res.results[i]["oe"]          # [2, NF] fp16
        f0 = 512 * i
        nf = 513 if i == NCORES - 1 else 512
        full = np.empty((2, 2048, NF), np.float32)
        gb0 = 0
        for B in GROUPS:
            seg = o[:, 32 * gb0:32 * gb0 + 32 * B].astype(np.float32)
            # [128, 32*B] -> [c, p, q, b] -> [c, 32p+q, b]
            full[:, :, gb0:gb0 + B] = \
                seg.reshape(2, 64, 32, B).reshape(2, 2048, B)
            gb0 += B
        out[:, :2048, f0:f0 + nf] = full[:, :, :nf]
        out[:, 2048, f0:f0 + nf] = oe[:, :nf].astype(np.float32)
    return out
